# revision 47
# baseline (speedup 1.0000x reference)
"""Trainium2 Bass kernel for nn_Attention_49907519980190 (v2: all-fp16).

Reference computation (b=2, n=2048, dim=1024, h=16, d=64):
    q = (x @ w_q)   -> (b, h, n, d)
    k, v = split(x @ w_vk)
    dots = (q @ k^T) * sqrt(d)          # multiplies by 8
    attn = softmax(dots)
    out = (attn @ v) -> (b, n, h*d) @ w_out

Sharding (8 cores): core c handles batch c // 4, heads 4*(c%4)..4*(c%4)+4.
Column-parallel q/k/v projections, row-parallel out projection; the host
sums the four partial outputs per batch.

v2 design vs the v1 baseline (479us):
- All-fp16 single-pass numerics (no bf16 hi/lo correction passes).
  Simulated end-to-end rel err 4.5e-3 vs the 2e-2 gate.
- Host supplies x^T fp16 (kills the on-chip PE transposes of x) and
  folds the *sqrt(d) scale into w_q, so dots PSUM holds final logits
  and the exp bias is just the negated row max.
- Q^T/K^T come out of the projection directly in dots layout
  ([d(2 heads stacked), tok]); dots contract K=64 per head.
- Softmax per 128-row block: 2x DVE max over [P,1024] PSUM halves
  (negated, combined with a tiny MIN reduce), 2x ACT exp -> u fp16,
  DMA-transpose u -> PV (ones column appended to V gives the
  denominator for free in PSUM col 64). Exp overflow is fatal (fp16
  exp yields inf, measured), so the row max must be exact.
- V projection and pair-1 Q/K projection chains are interleaved into
  the first attention sweep; per-token-tile out-projection + y DMA
  ride the second sweep. y is fp16; host accumulates fp32.
- 56 throwaway warm-up matmuls run during the input DMA so the PE HAM
  clock gate is at 8/8 when the projections start (88 measurably
  delayed the projections behind them in the PE FIFO).
PSUM: 3x[P,1024] S ring (6 banks) + 1 PV bank + 1 utility bank.

The dots stationary is Q^T zero-padded to K=128 (dead head rows = 0)
with the full 128-partition K^T as moving data: same timing per matmul,
but the whole PE array streams data, which keeps the HAM activity
monitor from clock-gating the PE to 4/8 (measured: stuck-cold without
this, oscillating warm with it).

Measured on 8 axon trn2 cores: ~352-364us cool-machine / ~414-424us
under thermal drift (v1 bf16-hi/lo baseline: ~479us, same machine),
rel err 4.5e-3 (gate 2e-2). Known remaining limiters (from NTFF traces):
the per-block DVE max (2x1.22us) + ACT exp (2x1.11us) chains with only
1.5 blocks of PSUM lookahead, and the PE HAM clock-gate dropping to
4/8 mid-run once the K=64 dots become the only PE work. Attempts that
measured WORSE and were reverted: 512-wide softmax quarters (+per-op
overhead), row-tiled concurrent head pairs (PSUM contention + 4-tiles-
on-3-slot ring), steady in-chain filler matmuls, PV-pop reordering
before the dots, out-proj transpose via DMA on the PT sync queue,
alternating PV accumulation between the psO/psX banks (hp0-late block
period went 4.7 -> 5.4us), and pipelining the first projections
c-outermost across 5 PSUM accumulators (fast but intermittent NaN —
unresolved race, do not retry without root-causing), and pure-WAW
filler matmuls into idle psX for HAM warmth (trace-neutral: hp0-late
period 5591 vs 5620 on matched hot-machine runs).
NOTE: the machine itself drifts ~±10% (thermal/P0 downclock shows up
as DVE reduce 1222 -> 1468ns in traces); compare configs only via
chronologically adjacent runs or trace-level period/head metrics.
"""

import numpy as np

import concourse.bass as bass
import concourse.mybir as mybir
import concourse.tile as tile
from concourse import bacc
from concourse.bass_utils import run_bass_kernel_spmd
from concourse.masks import make_identity

F32 = mybir.dt.float32
F16 = mybir.dt.float16
MAX = mybir.AluOpType.max
MIN = mybir.AluOpType.min
AX = mybir.AxisListType.X
EXP = mybir.ActivationFunctionType.Exp
COPY = mybir.ActivationFunctionType.Copy

P = 128      # partitions
NTOK = 2048  # tokens per core (one batch slice)
DIM = 1024   # model dim
E = 256      # per-core projection width (4 heads x 64)
NH = 4       # heads per core
D = 64       # head dim
D1 = 65      # head dim + ones column (denominator trick)
KO = 8       # contraction chunks of 128 over DIM
TT = 16      # token tiles of 128


def build_attention_nc():
    nc = bacc.Bacc("TRN2", target_bir_lowering=False, debug=False)

    xT = nc.declare_dram_parameter("xT", [DIM, NTOK], F16, isOutput=False)
    wq = nc.declare_dram_parameter("wq", [DIM, E], F16, isOutput=False)  # pre-scaled x8
    wk = nc.declare_dram_parameter("wk", [DIM, E], F16, isOutput=False)
    wv = nc.declare_dram_parameter("wv", [DIM, E], F16, isOutput=False)
    wo = nc.declare_dram_parameter("wo", [E, DIM], F16, isOutput=False)
    y = nc.declare_dram_parameter("y", [NTOK, DIM], F16, isOutput=True)

    with tile.TileContext(nc) as tc:
        with (
            tc.tile_pool(name="persist", bufs=1) as persist,
            tc.tile_pool(name="upool", bufs=3) as upool,
            tc.tile_pool(name="ptpool", bufs=10) as ptpool,
            tc.tile_pool(name="small", bufs=6) as small,
            tc.tile_pool(name="ysb", bufs=2) as ysb,
            tc.tile_pool(name="psS", bufs=3, space="PSUM") as psS,
            tc.tile_pool(name="psO", bufs=1, space="PSUM") as psO,
            tc.tile_pool(name="psX", bufs=1, space="PSUM") as psX,
        ):
            xTs = persist.tile([P, KO, NTOK], F16)
            wqs = persist.tile([P, KO, E], F16)
            wks = persist.tile([P, KO, E], F16)
            wvs = persist.tile([P, KO, E], F16)
            wos = persist.tile([P, 2, DIM], F16)
            # K^T: partition = d + 64*(h%2), free = (h//2, tok)
            KT = persist.tile([P, 2, NTOK], F16)
            # Q^T zero-padded to K=128 per head: head h occupies rows
            # 64*(h%2)..64*(h%2)+64 of plane h, the other 64 rows are zero.
            # The dots then stream the full 128-partition K^T as moving
            # data, keeping the whole PE array active (HAM clock gate).
            QTz = persist.tile([P, NH, NTOK], F16)
            # on gpsimd: the 8K-elem/partition memset took 6.9us on the DVE
            # queue right where the head-critical Q/K copies need it
            nc.gpsimd.memset(QTz[:, :, :], 0.0)
            # V natural layout + ones column per head
            Vb = persist.tile([P, TT, NH * D1], F16)
            Ob = persist.tile([P, TT, E], F16)
            identH = persist.tile([P, P], F16)
            make_identity(nc, identH)
            vb4 = Vb.rearrange("p t (h c) -> p t h c", c=D1)
            nc.vector.memset(vb4[:, :, :, D:D1], 1.0)

            # ---------------- input DMAs
            nc.sync.dma_start(
                out=wqs, in_=wq[:, :].rearrange("(ko p) e -> p ko e", p=P)
            )
            nc.sync.dma_start(
                out=wks, in_=wk[:, :].rearrange("(ko p) e -> p ko e", p=P)
            )
            for c in range(KO):
                eng = nc.gpsimd if c % 2 == 0 else nc.scalar
                eng.dma_start(out=xTs[:, c, :], in_=xT[c * P:(c + 1) * P, :])
            nc.sync.dma_start(
                out=wvs, in_=wv[:, :].rearrange("(ko p) e -> p ko e", p=P)
            )
            nc.sync.dma_start(
                out=wos, in_=wo[:, :].rearrange("(eo p) d -> p eo d", p=P)
            )

            # HAM warm-up: keep the PE busy while the x^T DMA streams in so
            # the clock gate is at 8/8 when the projections start. Results
            # are never read.
            warm = psX.tile([P, 512], F32, tag="X")
            for f in range(30):
                nc.tensor.matmul(
                    warm[:, :], wqs[:, f % KO, 0:P], xTs[:, 0, 0:512],
                    start=True, stop=True,
                )

            # ---------------- projections
            def proj_qk_chain(ws, dst, hp, g, copy_eng, pool, tag):
                """One [P,512] chain of Q^T/K^T for head-pair hp, token 512-group g."""
                pr = pool.tile([P, 512], F32, tag=tag)
                ms = slice(hp * P, (hp + 1) * P)
                gs = slice(g * 512, (g + 1) * 512)
                for c in range(KO):
                    nc.tensor.matmul(
                        pr[:, :], ws[:, c, ms], xTs[:, c, gs],
                        start=(c == 0), stop=(c == KO - 1),
                    )
                if dst is KT:
                    copy_eng(out=dst[:, hp, gs], in_=pr[:, :])
                else:
                    copy_eng(out=dst[0:64, 2 * hp, gs], in_=pr[0:64, :])
                    copy_eng(out=dst[64:128, 2 * hp + 1, gs], in_=pr[64:128, :])

            def issue_dots_softmax(h, it):
                isl = slice(it * P, (it + 1) * P)
                hp, hm = h // 2, h % 2
                halves = []
                m2 = small.tile([P, 2], F32, tag="m2")
                for nn in (0, 1):
                    S = psS.tile([P, 1024], F32, tag="S")
                    for q in (0, 1):
                        ns = slice(nn * 1024 + q * 512, nn * 1024 + (q + 1) * 512)
                        nc.tensor.matmul(
                            S[:, q * 512:(q + 1) * 512],
                            QTz[:, h, isl], KT[:, hp, ns],
                            start=True, stop=True,
                        )
                    nc.vector.tensor_reduce(
                        out=m2[:, nn:nn + 1], in_=S, axis=AX, op=MAX, negate=True,
                    )
                    halves.append(S)
                nm = small.tile([P, 1], F32, tag="nm")
                nc.vector.tensor_reduce(out=nm, in_=m2, axis=AX, op=MIN)
                u = upool.tile([P, NTOK], F16, tag="u")
                for nn in (0, 1):
                    cs = slice(nn * 1024, (nn + 1) * 1024)
                    nc.scalar.activation(
                        out=u[:, cs], in_=halves[nn], func=EXP, bias=nm,
                    )
                PT = ptpool.tile([P, TT, P], F16, tag="PT")
                nc.sync.dma_start_transpose(out=PT, in_=u)
                pending.append((h, it, PT))

            def proj_v_chain(tt, pool, tag):
                """V for all 4 heads of one 128-token tile."""
                pv = pool.tile([P, 512], F32, tag=tag)
                ts = slice(tt * P, (tt + 1) * P)
                for c in range(KO):
                    nc.tensor.matmul(
                        pv[:, 0:E], xTs[:, c, ts], wvs[:, c, :],
                        start=(c == 0), stop=(c == KO - 1),
                    )
                nc.scalar.copy(
                    out=vb4[:, tt, :, 0:D],
                    in_=pv[:, 0:E].rearrange("p (h d) -> p h d", d=D),
                )

            # Only the 5 chains the first attention block needs run up
            # front (all K^T groups + the first Q group); the remaining
            # pair-0 Q groups ride the first blocks like the V chains do.
            for g in range(4):
                proj_qk_chain(wks, KT, 0, g, nc.vector.tensor_copy, psS, "S")
            proj_qk_chain(wqs, QTz, 0, 0, nc.vector.tensor_copy, psS, "S")

            # ---------------- attention
            pending = []

            def issue_pv(h, it, PT, pool=None, tag="O"):
                pool = psO if pool is None else pool
                O_ps = pool.tile([P, D1], F32, tag=tag, name="O_ps")
                for jo in range(TT):
                    nc.tensor.matmul(
                        O_ps[:, :], PT[:, jo, :],
                        Vb[:, jo, h * D1:(h + 1) * D1],
                        start=(jo == 0), stop=(jo == TT - 1),
                    )
                rec = small.tile([P, 1], F32, tag="rec")
                nc.vector.reciprocal(out=rec, in_=O_ps[:, D:D1])
                nc.scalar.activation(
                    out=Ob[:, it, h * D:(h + 1) * D], in_=O_ps[:, :D],
                    func=COPY, scale=rec,
                )

            def issue_out(it):
                """y[it-tile] = Ob[it] @ wo, overlapped into the hp=1 sweep."""
                OT = small.tile([P, 2, P], F16, tag="OT", bufs=2)
                for eo in range(2):
                    ptp = psX.tile([P, P], F16, tag="X")
                    nc.tensor.transpose(
                        ptp[:, :], Ob[:, it, eo * P:(eo + 1) * P], identH[:, :]
                    )
                    nc.vector.tensor_copy(out=OT[:, eo, :], in_=ptp)
                yo = ysb.tile([P, DIM], F16, tag="yo")
                for n in range(2):
                    ns = slice(n * 512, (n + 1) * 512)
                    yp = psX.tile([P, 512], F32, tag="X")
                    for eo in range(2):
                        nc.tensor.matmul(
                            yp[:, :], OT[:, eo, :], wos[:, eo, ns],
                            start=(eo == 0), stop=(eo == 1),
                        )
                    nc.scalar.copy(out=yo[:, ns], in_=yp)
                nc.gpsimd.dma_start(
                    out=y[it * P:(it + 1) * P, :], in_=yo
                )

            # hp=0 sweep: V chains ride blocks 0..7 (2/tile), pair-1 Q/K
            # chains ride blocks 8..15 (1/block); PVs drain from block 8.
            step = 0
            for it in range(TT):
                for h in (0, 1):
                    issue_dots_softmax(h, it)
                    if step < 8:
                        if step < 3:
                            proj_qk_chain(
                                wqs, QTz, 0, step + 1,
                                nc.vector.tensor_copy, psS, "S",
                            )
                        proj_v_chain(2 * step, psX, "X")
                        proj_v_chain(2 * step + 1, psO, "O")
                    elif step < 12:
                        proj_qk_chain(wqs, QTz, 1, step - 8, nc.scalar.copy, psX, "X")
                        while len(pending) > 6:
                            issue_pv(*pending.pop(0))
                    elif step < 16:
                        proj_qk_chain(wks, KT, 1, step - 12, nc.scalar.copy, psX, "X")
                        while len(pending) > 5:
                            issue_pv(*pending.pop(0))
                    else:
                        while len(pending) > 3:
                            issue_pv(*pending.pop(0))
                    step += 1
            # hp=1 sweep with out-projection per token tile (lagged 2 tiles so
            # every PV writing Ob[it-2] is already emitted)
            for it in range(TT):
                for h in (2, 3):
                    issue_dots_softmax(h, it)
                    while len(pending) > 3:
                        issue_pv(*pending.pop(0))
                if it >= 2:
                    issue_out(it - 2)
            issue_pv(*pending.pop(0))
            issue_out(TT - 2)
            while pending:
                issue_pv(*pending.pop(0))
            issue_out(TT - 1)

    nc.compile()
    return nc


_NC_CACHE = None


def _get_nc():
    global _NC_CACHE
    if _NC_CACHE is None:
        _NC_CACHE = build_attention_nc()
    return _NC_CACHE


def kernel(x, w_q, w_vk, w_out, **run_kwargs):
    """Full inputs in, full output out. Shards over 8 NeuronCores."""
    b, n, dim = x.shape
    assert (b, n, dim) == (2, 2048, 1024)
    w_k = w_vk[:, :1024]
    w_v = w_vk[:, 1024:]

    xT = [np.ascontiguousarray(x[bi].T).astype(np.float16) for bi in range(2)]
    in_maps = []
    for c in range(8):
        bi = c // 4
        hg = c % 4
        cs = slice(hg * E, (hg + 1) * E)
        in_maps.append({
            "xT": xT[bi],
            "wq": np.ascontiguousarray(w_q[:, cs] * 8.0).astype(np.float16),
            "wk": np.ascontiguousarray(w_k[:, cs]).astype(np.float16),
            "wv": np.ascontiguousarray(w_v[:, cs]).astype(np.float16),
            "wo": np.ascontiguousarray(w_out[cs, :]).astype(np.float16),
        })

    nc = _get_nc()
    res = run_bass_kernel_spmd(nc, in_maps, core_ids=list(range(8)), **run_kwargs)
    out = np.zeros((2, 2048, 1024), dtype=np.float32)
    for c in range(8):
        out[c // 4] += res.results[c]["y"].astype(np.float32)
    if run_kwargs:
        kernel.last_results = res
    return out


# revision 50
# speedup vs baseline: 1.2039x; 1.2039x over previous
"""Trainium2 Bass kernel for nn_Attention_49907519980190 (v2: all-fp16).

Reference computation (b=2, n=2048, dim=1024, h=16, d=64):
    q = (x @ w_q)   -> (b, h, n, d)
    k, v = split(x @ w_vk)
    dots = (q @ k^T) * sqrt(d)          # multiplies by 8
    attn = softmax(dots)
    out = (attn @ v) -> (b, n, h*d) @ w_out

Sharding (8 cores): core c handles batch c // 4, heads 4*(c%4)..4*(c%4)+4.
Column-parallel q/k/v projections, row-parallel out projection; the host
sums the four partial outputs per batch.

v2 design vs the v1 baseline (479us):
- All-fp16 single-pass numerics (no bf16 hi/lo correction passes).
  Simulated end-to-end rel err 4.5e-3 vs the 2e-2 gate.
- Host supplies x^T fp16 (kills the on-chip PE transposes of x) and
  folds the *sqrt(d) scale into w_q, so dots PSUM holds final logits
  and the exp bias is just the negated row max.
- Q^T/K^T come out of the projection directly in dots layout
  ([d(2 heads stacked), tok]); dots contract K=64 per head.
- Softmax per 128-row block: 2x DVE max over [P,1024] PSUM halves
  (negated, combined with a tiny MIN reduce), 2x ACT exp -> u fp16,
  DMA-transpose u -> PV (ones column appended to V gives the
  denominator for free in PSUM col 64). Exp overflow is fatal (fp16
  exp yields inf, measured), so the row max must be exact.
- V projection and pair-1 Q/K projection chains are interleaved into
  the first attention sweep; per-token-tile out-projection + y DMA
  ride the second sweep. y is fp16; host accumulates fp32.
- 56 throwaway warm-up matmuls run during the input DMA so the PE HAM
  clock gate is at 8/8 when the projections start (88 measurably
  delayed the projections behind them in the PE FIFO).
PSUM: 3x[P,1024] S ring (6 banks) + 1 PV bank + 1 utility bank.

The dots stationary is Q^T zero-padded to K=128 (dead head rows = 0)
with the full 128-partition K^T as moving data: same timing per matmul,
but the whole PE array streams data, which keeps the HAM activity
monitor from clock-gating the PE to 4/8 (measured: stuck-cold without
this, oscillating warm with it).

Measured on 8 axon trn2 cores: ~352-364us cool-machine / ~414-424us
under thermal drift (v1 bf16-hi/lo baseline: ~479us, same machine),
rel err 4.5e-3 (gate 2e-2). Known remaining limiters (from NTFF traces):
the per-block DVE max (2x1.22us) + ACT exp (2x1.11us) chains with only
1.5 blocks of PSUM lookahead, and the PE HAM clock-gate dropping to
4/8 mid-run once the K=64 dots become the only PE work. Attempts that
measured WORSE and were reverted: 512-wide softmax quarters (+per-op
overhead), row-tiled concurrent head pairs (PSUM contention + 4-tiles-
on-3-slot ring), steady in-chain filler matmuls, PV-pop reordering
before the dots, out-proj transpose via DMA on the PT sync queue,
alternating PV accumulation between the psO/psX banks (hp0-late block
period went 4.7 -> 5.4us), and pipelining the first projections
c-outermost across 5 PSUM accumulators (fast but intermittent NaN —
unresolved race, do not retry without root-causing), and pure-WAW
filler matmuls into idle psX for HAM warmth (trace-neutral: hp0-late
period 5591 vs 5620 on matched hot-machine runs), and N=512 warm-up
matmuls streaming an x^T chunk (the chunk-DMA dep behind the gpsimd
memset delayed the first MM 15 -> 25us; head grew 42 -> 47us matched).
The warm-up must depend only on the FIRST sync-queue weight DMA.
NOTE: the machine itself drifts ~±10% (thermal/P0 downclock shows up
as DVE reduce 1222 -> 1468ns in traces); compare configs only via
chronologically adjacent runs or trace-level period/head metrics.
"""

import numpy as np

import concourse.bass as bass
import concourse.mybir as mybir
import concourse.tile as tile
from concourse import bacc
from concourse.bass_utils import run_bass_kernel_spmd
from concourse.masks import make_identity

F32 = mybir.dt.float32
F16 = mybir.dt.float16
MAX = mybir.AluOpType.max
MIN = mybir.AluOpType.min
AX = mybir.AxisListType.X
EXP = mybir.ActivationFunctionType.Exp
COPY = mybir.ActivationFunctionType.Copy

P = 128      # partitions
NTOK = 2048  # tokens per core (one batch slice)
DIM = 1024   # model dim
E = 256      # per-core projection width (4 heads x 64)
NH = 4       # heads per core
D = 64       # head dim
D1 = 65      # head dim + ones column (denominator trick)
KO = 8       # contraction chunks of 128 over DIM
TT = 16      # token tiles of 128


def build_attention_nc():
    nc = bacc.Bacc("TRN2", target_bir_lowering=False, debug=False)

    xT = nc.declare_dram_parameter("xT", [DIM, NTOK], F16, isOutput=False)
    wq = nc.declare_dram_parameter("wq", [DIM, E], F16, isOutput=False)  # pre-scaled x8
    wk = nc.declare_dram_parameter("wk", [DIM, E], F16, isOutput=False)
    wv = nc.declare_dram_parameter("wv", [DIM, E], F16, isOutput=False)
    wo = nc.declare_dram_parameter("wo", [E, DIM], F16, isOutput=False)
    y = nc.declare_dram_parameter("y", [NTOK, DIM], F16, isOutput=True)

    with tile.TileContext(nc) as tc:
        with (
            tc.tile_pool(name="persist", bufs=1) as persist,
            tc.tile_pool(name="upool", bufs=3) as upool,
            tc.tile_pool(name="ptpool", bufs=10) as ptpool,
            tc.tile_pool(name="small", bufs=6) as small,
            tc.tile_pool(name="ysb", bufs=2) as ysb,
            tc.tile_pool(name="psS", bufs=3, space="PSUM") as psS,
            tc.tile_pool(name="psO", bufs=1, space="PSUM") as psO,
            tc.tile_pool(name="psX", bufs=1, space="PSUM") as psX,
        ):
            xTs = persist.tile([P, KO, NTOK], F16)
            wqs = persist.tile([P, KO, E], F16)
            wks = persist.tile([P, KO, E], F16)
            wvs = persist.tile([P, KO, E], F16)
            wos = persist.tile([P, 2, DIM], F16)
            # K^T: partition = d + 64*(h%2), free = (h//2, tok)
            KT = persist.tile([P, 2, NTOK], F16)
            # Q^T zero-padded to K=128 per head: head h occupies rows
            # 64*(h%2)..64*(h%2)+64 of plane h, the other 64 rows are zero.
            # The dots then stream the full 128-partition K^T as moving
            # data, keeping the whole PE array active (HAM clock gate).
            QTz = persist.tile([P, NH, NTOK], F16)
            # on gpsimd: the 8K-elem/partition memset took 6.9us on the DVE
            # queue right where the head-critical Q/K copies need it
            nc.gpsimd.memset(QTz[:, :, :], 0.0)
            # V natural layout + ones column per head
            Vb = persist.tile([P, TT, NH * D1], F16)
            Ob = persist.tile([P, TT, E], F16)
            identH = persist.tile([P, P], F16)
            make_identity(nc, identH)
            vb4 = Vb.rearrange("p t (h c) -> p t h c", c=D1)
            nc.vector.memset(vb4[:, :, :, D:D1], 1.0)

            # ---------------- input DMAs
            nc.sync.dma_start(
                out=wqs, in_=wq[:, :].rearrange("(ko p) e -> p ko e", p=P)
            )
            nc.sync.dma_start(
                out=wks, in_=wk[:, :].rearrange("(ko p) e -> p ko e", p=P)
            )
            for c in range(KO):
                eng = nc.gpsimd if c % 2 == 0 else nc.scalar
                eng.dma_start(out=xTs[:, c, :], in_=xT[c * P:(c + 1) * P, :])
            nc.sync.dma_start(
                out=wvs, in_=wv[:, :].rearrange("(ko p) e -> p ko e", p=P)
            )
            nc.sync.dma_start(
                out=wos, in_=wo[:, :].rearrange("(eo p) d -> p eo d", p=P)
            )

            # HAM warm-up: keep the PE busy while the x^T DMA streams in so
            # the clock gate is at 8/8 when the projections start. Results
            # are never read.
            warm = psX.tile([P, 512], F32, tag="X")
            for f in range(30):
                c2 = 2 * (f % 4)
                nc.tensor.matmul(
                    warm[:, :], wqs[:, f % KO, 0:P], wqs[:, c2:c2 + 2, :],
                    start=True, stop=True,
                )

            # ---------------- projections
            def proj_qk_chain(ws, dst, hp, g, copy_eng, pool, tag):
                """One [P,512] chain of Q^T/K^T for head-pair hp, token 512-group g."""
                pr = pool.tile([P, 512], F32, tag=tag)
                ms = slice(hp * P, (hp + 1) * P)
                gs = slice(g * 512, (g + 1) * 512)
                for c in range(KO):
                    nc.tensor.matmul(
                        pr[:, :], ws[:, c, ms], xTs[:, c, gs],
                        start=(c == 0), stop=(c == KO - 1),
                    )
                if dst is KT:
                    copy_eng(out=dst[:, hp, gs], in_=pr[:, :])
                else:
                    copy_eng(out=dst[0:64, 2 * hp, gs], in_=pr[0:64, :])
                    copy_eng(out=dst[64:128, 2 * hp + 1, gs], in_=pr[64:128, :])

            def issue_dots_softmax(h, it):
                isl = slice(it * P, (it + 1) * P)
                hp, hm = h // 2, h % 2
                halves = []
                m2 = small.tile([P, 2], F32, tag="m2")
                for nn in (0, 1):
                    S = psS.tile([P, 1024], F32, tag="S")
                    for q in (0, 1):
                        ns = slice(nn * 1024 + q * 512, nn * 1024 + (q + 1) * 512)
                        nc.tensor.matmul(
                            S[:, q * 512:(q + 1) * 512],
                            QTz[:, h, isl], KT[:, hp, ns],
                            start=True, stop=True,
                        )
                    nc.vector.tensor_reduce(
                        out=m2[:, nn:nn + 1], in_=S, axis=AX, op=MAX, negate=True,
                    )
                    halves.append(S)
                nm = small.tile([P, 1], F32, tag="nm")
                nc.vector.tensor_reduce(out=nm, in_=m2, axis=AX, op=MIN)
                u = upool.tile([P, NTOK], F16, tag="u")
                for nn in (0, 1):
                    cs = slice(nn * 1024, (nn + 1) * 1024)
                    nc.scalar.activation(
                        out=u[:, cs], in_=halves[nn], func=EXP, bias=nm,
                    )
                PT = ptpool.tile([P, TT, P], F16, tag="PT")
                nc.sync.dma_start_transpose(out=PT, in_=u)
                pending.append((h, it, PT))

            def proj_v_chain(tt, pool, tag):
                """V for all 4 heads of one 128-token tile."""
                pv = pool.tile([P, 512], F32, tag=tag)
                ts = slice(tt * P, (tt + 1) * P)
                for c in range(KO):
                    nc.tensor.matmul(
                        pv[:, 0:E], xTs[:, c, ts], wvs[:, c, :],
                        start=(c == 0), stop=(c == KO - 1),
                    )
                nc.scalar.copy(
                    out=vb4[:, tt, :, 0:D],
                    in_=pv[:, 0:E].rearrange("p (h d) -> p h d", d=D),
                )

            # Only the 5 chains the first attention block needs run up
            # front (all K^T groups + the first Q group); the remaining
            # pair-0 Q groups ride the first blocks like the V chains do.
            for g in range(4):
                proj_qk_chain(wks, KT, 0, g, nc.vector.tensor_copy, psS, "S")
            proj_qk_chain(wqs, QTz, 0, 0, nc.vector.tensor_copy, psS, "S")

            # ---------------- attention
            pending = []

            def issue_pv(h, it, PT, pool=None, tag="O"):
                pool = psO if pool is None else pool
                O_ps = pool.tile([P, D1], F32, tag=tag, name="O_ps")
                for jo in range(TT):
                    nc.tensor.matmul(
                        O_ps[:, :], PT[:, jo, :],
                        Vb[:, jo, h * D1:(h + 1) * D1],
                        start=(jo == 0), stop=(jo == TT - 1),
                    )
                rec = small.tile([P, 1], F32, tag="rec")
                nc.vector.reciprocal(out=rec, in_=O_ps[:, D:D1])
                nc.scalar.activation(
                    out=Ob[:, it, h * D:(h + 1) * D], in_=O_ps[:, :D],
                    func=COPY, scale=rec,
                )

            def issue_out(it):
                """y[it-tile] = Ob[it] @ wo, overlapped into the hp=1 sweep."""
                OT = small.tile([P, 2, P], F16, tag="OT", bufs=2)
                for eo in range(2):
                    ptp = psX.tile([P, P], F16, tag="X")
                    nc.tensor.transpose(
                        ptp[:, :], Ob[:, it, eo * P:(eo + 1) * P], identH[:, :]
                    )
                    nc.vector.tensor_copy(out=OT[:, eo, :], in_=ptp)
                yo = ysb.tile([P, DIM], F16, tag="yo")
                for n in range(2):
                    ns = slice(n * 512, (n + 1) * 512)
                    yp = psX.tile([P, 512], F32, tag="X")
                    for eo in range(2):
                        nc.tensor.matmul(
                            yp[:, :], OT[:, eo, :], wos[:, eo, ns],
                            start=(eo == 0), stop=(eo == 1),
                        )
                    nc.scalar.copy(out=yo[:, ns], in_=yp)
                nc.gpsimd.dma_start(
                    out=y[it * P:(it + 1) * P, :], in_=yo
                )

            # hp=0 sweep: V chains ride blocks 0..7 (2/tile), pair-1 Q/K
            # chains ride blocks 8..15 (1/block); PVs drain from block 8.
            step = 0
            for it in range(TT):
                for h in (0, 1):
                    issue_dots_softmax(h, it)
                    if step < 8:
                        if step < 3:
                            proj_qk_chain(
                                wqs, QTz, 0, step + 1,
                                nc.vector.tensor_copy, psS, "S",
                            )
                        proj_v_chain(2 * step, psX, "X")
                        proj_v_chain(2 * step + 1, psO, "O")
                    elif step < 12:
                        proj_qk_chain(wqs, QTz, 1, step - 8, nc.scalar.copy, psX, "X")
                        while len(pending) > 6:
                            issue_pv(*pending.pop(0))
                    elif step < 16:
                        proj_qk_chain(wks, KT, 1, step - 12, nc.scalar.copy, psX, "X")
                        while len(pending) > 5:
                            issue_pv(*pending.pop(0))
                    else:
                        while len(pending) > 3:
                            issue_pv(*pending.pop(0))
                    step += 1
            # hp=1 sweep with out-projection per token tile (lagged 2 tiles so
            # every PV writing Ob[it-2] is already emitted)
            for it in range(TT):
                for h in (2, 3):
                    issue_dots_softmax(h, it)
                    while len(pending) > 3:
                        issue_pv(*pending.pop(0))
                if it >= 2:
                    issue_out(it - 2)
            issue_pv(*pending.pop(0))
            issue_out(TT - 2)
            while pending:
                issue_pv(*pending.pop(0))
            issue_out(TT - 1)

    nc.compile()
    return nc


_NC_CACHE = None


def _get_nc():
    global _NC_CACHE
    if _NC_CACHE is None:
        _NC_CACHE = build_attention_nc()
    return _NC_CACHE


def kernel(x, w_q, w_vk, w_out, **run_kwargs):
    """Full inputs in, full output out. Shards over 8 NeuronCores."""
    b, n, dim = x.shape
    assert (b, n, dim) == (2, 2048, 1024)
    w_k = w_vk[:, :1024]
    w_v = w_vk[:, 1024:]

    xT = [np.ascontiguousarray(x[bi].T).astype(np.float16) for bi in range(2)]
    in_maps = []
    for c in range(8):
        bi = c // 4
        hg = c % 4
        cs = slice(hg * E, (hg + 1) * E)
        in_maps.append({
            "xT": xT[bi],
            "wq": np.ascontiguousarray(w_q[:, cs] * 8.0).astype(np.float16),
            "wk": np.ascontiguousarray(w_k[:, cs]).astype(np.float16),
            "wv": np.ascontiguousarray(w_v[:, cs]).astype(np.float16),
            "wo": np.ascontiguousarray(w_out[cs, :]).astype(np.float16),
        })

    nc = _get_nc()
    res = run_bass_kernel_spmd(nc, in_maps, core_ids=list(range(8)), **run_kwargs)
    out = np.zeros((2, 2048, 1024), dtype=np.float32)
    for c in range(8):
        out[c // 4] += res.results[c]["y"].astype(np.float32)
    if run_kwargs:
        kernel.last_results = res
    return out
